# revision 58
# baseline (speedup 1.0000x reference)
"""Multi-head attention (B=4, S=2048, D=1024, H=16) on 8 trn2 NeuronCores.

Sharding: core = (batch b, head-group g) with b = core//2, g = core%2.
Each core handles one batch and 8 heads (512 of the 1024 d_model dims):
  - host pre-transposes query/key/value[b] -> [1024, 2048] so the device
    never transposes activations (and pre-casts to the matmul dtype)
  - device computes Q^T, K^T (head dims on partitions) and V (natural),
    attention with *transposed* scores S^T = K_h @ Q_h^T so softmax's
    denominator comes out of the PV matmul via a ones-column appended to V
  - output projection vs Wo[g*512:(g+1)*512, :] gives a partial [2048,1024]
  - host sums the two group partials per batch and adds bv@Wo + bo
Matmul operand dtype is MM_DT (bf16 default: full-rate PE streaming + FWL;
f32r fallback: fp22 multiplies at half stream rate). PSUM accumulation and
the softmax normalization chain stay fp32.
"""

import os
import numpy as np
from contextlib import ExitStack

B = 4
S = 2048
D = 1024
H = 16
DK = 64
NCORES = 8
GH = 8          # heads per core (group)
GD = GH * DK    # 512 head dims per core
NCH = GD // 128  # 4 chunks of 128 output dims
KT = S // 128    # 16 key tiles
QC = 1024        # q chunk width for attention
NQC = S // QC    # 2
SC = 512         # s chunk width for projections
NSC = S // SC    # 8
DMT = D // 128   # 8 d_model tiles

MM_DT = os.environ.get("MM_DT", "bf16")  # "bf16" | "f32r"

_CACHE = {}


def _np_mm_dtype():
    if MM_DT == "bf16":
        import ml_dtypes
        return ml_dtypes.bfloat16
    return np.float32


def _build_program():
    import concourse.mybir as mybir
    import concourse.tile as tile
    from concourse import bacc

    f32 = mybir.dt.float32
    f32r = mybir.dt.float32r
    dmm = mybir.dt.bfloat16 if MM_DT == "bf16" else mybir.dt.float32r

    nc = bacc.Bacc("TRN2", target_bir_lowering=False, debug=False,
                   num_devices=NCORES)

    xqT = nc.dram_tensor("xqT", [D, S], dmm, kind="ExternalInput").ap()
    xkT = nc.dram_tensor("xkT", [D, S], dmm, kind="ExternalInput").ap()
    xvT = nc.dram_tensor("xvT", [D, S], dmm, kind="ExternalInput").ap()
    wq = nc.dram_tensor("wq", [D, GD], dmm, kind="ExternalInput").ap()
    wk = nc.dram_tensor("wk", [D, GD], dmm, kind="ExternalInput").ap()
    wv = nc.dram_tensor("wv", [D, GD], dmm, kind="ExternalInput").ap()
    wo = nc.dram_tensor("wo", [GD, D], dmm, kind="ExternalInput").ap()
    bq = nc.dram_tensor("bq", [GD], f32, kind="ExternalInput").ap()
    bk = nc.dram_tensor("bk", [GD], f32, kind="ExternalInput").ap()
    out = nc.dram_tensor("out", [S, D], dmm, kind="ExternalOutput").ap()

    dbg = os.environ.get("DEBUG_DUMPS", "0") == "1"
    if dbg:
        d_qt = nc.dram_tensor("d_qt", [128, S], dmm, kind="ExternalOutput").ap()
        d_kt = nc.dram_tensor("d_kt", [128, S], dmm, kind="ExternalOutput").ap()
        d_v = nc.dram_tensor("d_v", [128, GH * 66], dmm,
                             kind="ExternalOutput").ap()
        d_pt = nc.dram_tensor("d_pt", [128, QC], dmm,
                              kind="ExternalOutput").ap()
        d_pv = nc.dram_tensor("d_pv", [65, QC], f32, kind="ExternalOutput").ap()
        d_zr = nc.dram_tensor("d_zr", [1, QC], f32, kind="ExternalOutput").ap()
        d_rb = nc.dram_tensor("d_rb", [DK, QC], f32, kind="ExternalOutput").ap()
        d_ot = nc.dram_tensor("d_ot", [128, S], dmm, kind="ExternalOutput").ap()

    Exp = mybir.ActivationFunctionType.Exp

    with tile.TileContext(nc) as tc, ExitStack() as ctx:
        # ---- pools (slots are statically reserved per tag) ----
        p_qt = ctx.enter_context(tc.tile_pool(name="qt", bufs=GH))
        p_kt = ctx.enter_context(tc.tile_pool(name="kt", bufs=GH))
        p_v = ctx.enter_context(tc.tile_pool(name="v", bufs=KT))
        p_ot = ctx.enter_context(tc.tile_pool(name="ot", bufs=NCH))
        p_wvo = ctx.enter_context(tc.tile_pool(name="wvo", bufs=1))
        p_wc = ctx.enter_context(tc.tile_pool(name="wc", bufs=1))
        p_bias = ctx.enter_context(tc.tile_pool(name="bias", bufs=1))
        p_xs = ctx.enter_context(tc.tile_pool(name="xs", bufs=3))
        p_pt = ctx.enter_context(tc.tile_pool(name="pt", bufs=10))
        p_zr = ctx.enter_context(tc.tile_pool(name="zr", bufs=2))
        p_rb = ctx.enter_context(tc.tile_pool(name="rb", bufs=2))
        p_ov = ctx.enter_context(tc.tile_pool(name="ov", bufs=2))
        p_st = ctx.enter_context(tc.tile_pool(name="st", bufs=3))
        # PSUM (8 banks total): scores 2x[128,1024]f32 (4) + proj
        # 2x[128,512]f32 (2) + PV accum 1x[65,1024]f32 (2). A separate proj
        # pool keeps scores-psum recycling off the DVE bias-add chain.
        p_ps = ctx.enter_context(tc.tile_pool(name="ps", bufs=2, space="PSUM"))
        p_pj = ctx.enter_context(tc.tile_pool(name="pj", bufs=2, space="PSUM"))
        p_pv = ctx.enter_context(tc.tile_pool(name="pv", bufs=1, space="PSUM"))

        # ---- biases + ones ----
        bq_sb = p_bias.tile([128, NCH], f32, tag="bq")
        nc.sync.dma_start(out=bq_sb[:], in_=bq.rearrange("(a p) -> p a", p=128))
        bk_sb = p_bias.tile([128, NCH], f32, tag="bk")
        nc.sync.dma_start(out=bk_sb[:], in_=bk.rearrange("(a p) -> p a", p=128))
        ones_sb = p_bias.tile([128, 1], f32, tag="ones")
        nc.vector.memset(ones_sb[:], 1.0)


        # ---- V projection: V_sb[st] = [128 s, GH, 65] (col 64 = ones) ----
        v_sb = []

        vstate = {}

        def v_filler(st):
            def emit():
                emit_v_st(st)
            return emit

        def emit_v_proj(first=0):
          wv_sb = p_wvo.tile([128, DMT, GD], dmm, tag="wvo", name="wv_sb")
          nc.scalar.dma_start(out=wv_sb[:],
                              in_=wv.rearrange("(a p) d -> p a d", p=128))
          vstate["wv"] = wv_sb
          for st in range(first):
              emit_v_st(st)

        def emit_v_st(st):
          wv_sb = vstate["wv"]
          if True:
              if st % 4 == 0:
                  xv_t = p_xs.tile([128, DMT, 512], dmm, tag="xs",
                                   name=f"xv{st}")
                  nc.sync.dma_start(
                      out=xv_t[:],
                      in_=xvT[:, st * 128:st * 128 + 512].rearrange(
                          "(a p) s -> p a s", p=128),
                  )
                  vstate["xv"] = xv_t
              xv_t = vstate["xv"]
              sub = (st % 4) * 128
              ps = p_pj.tile([128, GD], f32, tag="pj", name=f"psv{st}")
              for a in range(DMT):
                  nc.tensor.matmul(
                      out=ps[:],
                      lhsT=xv_t[:, a, sub:sub + 128],
                      rhs=wv_sb[:, a, :],
                      start=(a == 0), stop=(a == DMT - 1),
                  )
              # layout [ones | V(64) | ones]: even heads take cols 1:66
              # (Z lands at psum row 64), odd heads cols 0:65 written at
              # psum rows 63:128 (Z at row 63) -> both land row-aligned
              # with their ot half, so normalize needs no partition shift
              vt = p_v.tile([128, GH, 66], dmm, tag="v", name=f"v{st}")
              nc.vector.tensor_copy(
                  out=vt[:, :, 1:1 + DK],
                  in_=ps[:].rearrange("p (h d) -> p h d", h=GH),
              )
              nc.vector.tensor_copy(
                  out=vt[:, :, 0:1],
                  in_=ones_sb.unsqueeze(1).broadcast_to([128, GH, 1]))
              nc.vector.tensor_copy(
                  out=vt[:, :, 65:66],
                  in_=ones_sb.unsqueeze(1).broadcast_to([128, GH, 1]))
              v_sb.append(vt)

        qt_sb = [None] * GH
        kt_sb = [None] * GH
        ot_sb = [None] * NCH
        # deferred trailing PVs + normalize of the previous (qc,hh) loop;
        # they re-emit after the NEXT loop's first scores (cross-pair too)
        pend = {"d": []}

        wq_sb = p_wc.tile([128, DMT, GD], dmm, tag="wqc", name="wq_sb")
        nc.scalar.dma_start(out=wq_sb[:],
                          in_=wq.rearrange("(a p) d -> p a d", p=128))
        wk_sb = p_wc.tile([128, DMT, GD], dmm, tag="wkc", name="wk_sb")
        nc.scalar.dma_start(out=wk_sb[:],
                          in_=wk.rearrange("(a p) d -> p a d", p=128))

        def proj_fillers(c):
            """Emission groups computing per-head Q^T/K^T for heads 2c,2c+1.
            Each head tile [128, S] holds its 64 dims twice (rows 0-63 and
            64-127) so consecutive kt score matmuls alternate PE row groups
            and run concurrently."""
            for hh in range(2):
                hg = 2 * c + hh
                qt_sb[hg] = p_qt.tile([128, S], dmm, tag="qt", name=f"qt{hg}")
                kt_sb[hg] = p_kt.tile([128, S], dmm, tag="kt", name=f"kt{hg}")

            def group(src, wsb, bsb, dsts, nm, sc):
                def emit():
                    xs = p_xs.tile([128, DMT, SC], dmm, tag="xs",
                                   name=f"xs{nm}{c}_{sc}")
                    # k-chunks on the sync queue, q-chunks on gpsimd: the
                    # startup-critical loads stream on separate queues
                    dq = nc.sync if nm == "k" else nc.gpsimd
                    dq.dma_start(
                        out=xs[:],
                        in_=src[:, sc * SC:(sc + 1) * SC].rearrange(
                            "(a p) s -> p a s", p=128),
                    )
                    ps = p_pj.tile([128, SC], f32, tag="pj",
                                   name=f"psp{nm}{c}_{sc}")
                    for a in range(DMT):
                        nc.tensor.matmul(
                            out=ps[:],
                            lhsT=wsb[:, a, c * 128:(c + 1) * 128],
                            rhs=xs[:, a, :],
                            start=(a == 0), stop=(a == DMT - 1),
                        )
                    s0, s1 = sc * SC, (sc + 1) * SC
                    # head 2c native rows 0-63; head 2c+1 native rows 64-127
                    nc.vector.tensor_scalar_add(
                        out=dsts[0][0:DK, s0:s1], in0=ps[0:DK, :],
                        scalar1=bsb[0:DK, c:c + 1])
                    nc.vector.tensor_scalar_add(
                        out=dsts[1][DK:128, s0:s1], in0=ps[DK:128, :],
                        scalar1=bsb[DK:128, c:c + 1])
                    # duplicate this slice into the other half right away
                    # (SBUF->SBUF DMA) so scores kt for this s-range unblock
                    nc.sync.dma_start(out=dsts[0][DK:128, s0:s1],
                                      in_=dsts[0][0:DK, s0:s1])
                    nc.sync.dma_start(out=dsts[1][0:DK, s0:s1],
                                      in_=dsts[1][DK:128, s0:s1])
                return emit

            qd = [qt_sb[2 * c], qt_sb[2 * c + 1]]
            kd = [kt_sb[2 * c], kt_sb[2 * c + 1]]
            q = lambda sc: group(xqT, wq_sb, bq_sb, qd, "q", sc)
            k = lambda sc: group(xkT, wk_sb, bk_sb, kd, "k", sc)
            # K sc0 + Q sc0/1 first: pair c's scores kt=0 needs them
            return [k(0), q(0), q(1), k(1), k(2), q(2), k(3), q(3)]

        def attention_pair(c, fillers=(), pace=4, slow_fillers=(),
                           qc1_fillers=(), fin_cb=None):
            """Heads 2c, 2c+1 -> normalized O^T chunk c [128 dout, S].
            fillers: emission callbacks interleaved into the kt loop so
            next-chunk projections share PE/PSUM without starving ACT."""
            fillers = list(fillers)
            slow_fillers = list(slow_fillers)
            qc1_fillers = list(qc1_fillers)

            def normalize_half(qc, hh, pv_ps, half, direct):
                """per-512-half normalize chain (for the last loop, so
                dependent final projections can start after each half)"""
                sl = slice(half * 512, (half + 1) * 512)
                osl = slice(qc * QC + half * 512, qc * QC + (half + 1) * 512)
                ov = p_ov.tile([65, QC], f32, tag="ov",
                               name=f"ovh{c}_{qc}_{hh}_{half}")
                nc.vector.tensor_copy(out=ov[:, 0:512], in_=pv_ps[0:65, sl])
                zs = p_zr.tile([DK, QC // DK], f32, tag="zs",
                               name=f"zsh{c}_{qc}_{hh}_{half}")
                nc.sync.dma_start(out=zs[:, 0:8], in_=ov[DK:DK + 1, 0:512])
                nc.vector.reciprocal(out=zs[:, 0:8], in_=zs[:, 0:8])
                zr = p_zr.tile([1, QC], f32, tag="zr",
                               name=f"zrh{c}_{qc}_{hh}_{half}")
                nc.sync.dma_start(out=zr[:, 0:512], in_=zs[:, 0:8])
                rb = p_rb.tile([DK, QC], f32, tag="rb",
                               name=f"rbh{c}_{qc}_{hh}_{half}")
                nc.gpsimd.partition_broadcast(rb[:, 0:512], zr[:, 0:512],
                                              channels=DK)
                if direct:
                    nc.vector.tensor_mul(out=ot_sb[c][0:DK, osl],
                                         in0=ov[0:DK, 0:512],
                                         in1=rb[:, 0:512])
                else:
                    tmp = p_rb.tile([DK, QC], dmm, tag="tmp",
                                    name=f"tmph{c}_{qc}_{hh}_{half}")
                    nc.vector.tensor_mul(out=tmp[:, 0:512],
                                         in0=ov[0:DK, 0:512],
                                         in1=rb[:, 0:512])
                    nc.sync.dma_start(out=ot_sb[c][DK:128, osl],
                                      in_=tmp[:, 0:512])
            def normalize_full(qc, hh, pv_ps, direct):
                # normalize: DVE evicts PV psum (frees the bank pair
                # fast), takes 1/Z via scatter (64-wide; single-lane DVE
                # recip is ~6x slower), gpsimd broadcasts, DVE multiplies
                ov = p_ov.tile([65, QC], f32, tag="ov",
                               name=f"ov{c}_{qc}_{hh}")
                nc.vector.tensor_copy(out=ov[:], in_=pv_ps[0:65, :])
                zs = p_zr.tile([DK, QC // DK], f32, tag="zs",
                               name=f"zs{c}_{qc}_{hh}")
                nc.sync.dma_start(out=zs[:], in_=ov[DK:DK + 1, :])
                nc.vector.reciprocal(out=zs[:], in_=zs[:])
                zr = p_zr.tile([1, QC], f32, tag="zr",
                               name=f"zr{c}_{qc}_{hh}")
                nc.sync.dma_start(out=zr[:], in_=zs[:])
                rb = p_rb.tile([DK, QC], f32, tag="rb",
                               name=f"rb{c}_{qc}_{hh}")
                nc.gpsimd.partition_broadcast(rb[:], zr[:], channels=DK)
                if direct:
                    nc.vector.tensor_mul(
                        out=ot_sb[c][0:DK, qc * QC:(qc + 1) * QC],
                        in0=ov[0:DK, :], in1=rb[:])
                else:
                    tmp = p_rb.tile([DK, QC], dmm, tag="tmp",
                                    name=f"tmp{c}_{qc}")
                    nc.vector.tensor_mul(out=tmp[:], in0=ov[0:DK, :],
                                         in1=rb[:])
                    nc.sync.dma_start(
                        out=ot_sb[c][DK:128, qc * QC:(qc + 1) * QC],
                        in_=tmp[:])

            ot_sb[c] = p_ot.tile([128, S], dmm, tag="ot", name=f"ot{c}")
            for qc in range(NQC):
                for hh in range(2):
                    hg = 2 * c + hh
                    # PV -> psum rows 0:65 (num 0:64, Z at row 64). One head
                    # of the pair writes ot rows 0:64 directly; the other is
                    # shifted to rows 64:128 by an SBUF DMA. For pair 3 the
                    # hh processed LAST gets the direct write (no dup on the
                    # kernel's critical tail) -- the host swaps Wo's c=3 row
                    # blocks to match.
                    direct = (hh == 1) if c == 3 else (hh == 0)
                    last = fin_cb is not None and hh == 1 and qc == NQC - 1
                    pv_ps = p_pv.tile([128, QC], f32, tag="pv",
                                      name=f"pv{c}_{qc}_{hh}")

                    def emit_se(kt_i, qc=qc, hh=hh, hg=hg):
                        """scores + exp for kt_i -> P^T tile."""
                        rg = DK * (kt_i % 2)
                        ps = p_ps.tile([128, QC], f32, tag="ps",
                                       name=f"pss{c}_{qc}_{kt_i}_{hh}")
                        for half in range(QC // 512):
                            q0 = qc * QC + half * 512
                            nc.tensor.matmul(
                                out=ps[:, half * 512:(half + 1) * 512],
                                lhsT=kt_sb[hg][rg:rg + DK,
                                               kt_i * 128:(kt_i + 1) * 128],
                                rhs=qt_sb[hg][rg:rg + DK, q0:q0 + 512],
                                start=True, stop=True,
                            )
                        pt = p_pt.tile([128, QC], dmm, tag="pt",
                                       name=f"pt{c}_{qc}_{kt_i}_{hh}")
                        nc.scalar.activation(pt[:], ps[:], Exp,
                                             bias=0.0, scale=0.125)
                        return pt

                    # software pipeline: scores/exp run PIPE kts ahead of
                    # PV; the previous loop's trailing PVs + normalize are
                    # deferred until after this loop's first scores so the
                    # exp stream is seamless across loop boundaries
                    PIPE = 2
                    pts = [emit_se(i) for i in range(PIPE)]
                    for d_ in pend["d"]:
                        d_()
                    pend["d"] = []

                    def emit_pv(kt_i, pv_ps=pv_ps, hg=hg, pts=pts):
                        pt_cur = pts[kt_i]
                        for half in range(QC // 512):
                            nc.tensor.matmul(
                                out=pv_ps[0:65,
                                          half * 512:(half + 1) * 512],
                                lhsT=v_sb[kt_i][:, hg, 1:66],
                                rhs=pt_cur[:, half * 512:(half + 1) * 512],
                                start=(kt_i == 0), stop=(kt_i == KT - 1),
                            )

                    for kt_i in range(KT):
                        if kt_i + PIPE < KT:
                            pts.append(emit_se(kt_i + PIPE))
                        # pops at kt 1,5,9,12 -- never in the last 3 kts,
                        # where a filler would sit between this loop's tail
                        # PVs and the next loop's scores and drain the exp
                        # pipeline at every boundary
                        if fillers and (pace == 1 or kt_i in (1, 5, 9, 12)):
                            fillers.pop(0)()
                        elif slow_fillers and kt_i in (1, 5, 9, 12):
                            slow_fillers.pop(0)()
                        elif qc == 1 and qc1_fillers and kt_i in (1, 5, 9, 12):
                            qc1_fillers.pop(0)()
                        emit_pv(kt_i)
                    if last:
                        # last loop of the whole kernel: normalize per half
                        # so the trailing final projections start early
                        for half in range(QC // 512):
                            normalize_half(qc, hh, pv_ps, half, direct)
                            fin_cb(half)
                        continue
                    normalize_full(qc, hh, pv_ps, direct)


        # ---- output projection ----
        def emit_final(qts, use_act=False):
          Copy = mybir.ActivationFunctionType.Copy
          for qt_i in qts:
              st = p_st.tile([128, D], dmm, tag="st", name=f"st{qt_i}")
              if use_act:
                  # exp-free tail: borrow the idle scores pool (2-bank
                  # tiles) so 4 psum slots rotate, and let ACT drain them
                  ps = p_ps.tile([128, 1024], f32, tag="ps",
                                 name=f"pso{qt_i}")
                  for half in range(2):
                      sl = slice(half * 512, (half + 1) * 512)
                      for c in range(NCH):
                          nc.tensor.matmul(
                              out=ps[:, sl],
                              lhsT=ot_sb[c][:, qt_i * 128:(qt_i + 1) * 128],
                              rhs=wo_sb[:, c, sl],
                              start=(c == 0), stop=(c == NCH - 1),
                          )
                      nc.scalar.activation(st[:, sl], ps[:, sl], Copy)
              else:
                  for half in range(2):
                      sl = slice(half * 512, (half + 1) * 512)
                      ps = p_pj.tile([128, 512], f32, tag="pj",
                                     name=f"pso{qt_i}_{half}")
                      for c in range(NCH):
                          nc.tensor.matmul(
                              out=ps[:],
                              lhsT=ot_sb[c][:, qt_i * 128:(qt_i + 1) * 128],
                              rhs=wo_sb[:, c, sl],
                              start=(c == 0), stop=(c == NCH - 1),
                          )
                      nc.vector.tensor_copy(out=st[:, sl], in_=ps[:])
              nc.sync.dma_start(out=out[qt_i * 128:(qt_i + 1) * 128, :],
                                in_=st[:])

        # ---- emit: QK chunk 0 first so attention starts ASAP; V proj
        # streams in behind it; later chunk projections fill PE gaps ----
        wo_sb = p_wvo.tile([128, NCH, D], dmm, tag="wo", name="wo_sb")
        g0 = proj_fillers(0)
        for g in g0[:3]:        # k0, q0, q1 -> first scores ready ASAP
            g()
        k1, k2, q2, k3, q3 = g0[3], g0[4], g0[5], g0[6], g0[7]
        k1()                    # loop0 scores reach kt4 ~4 exps in
        emit_v_proj(first=2)    # wv + v0,v1; v2+ stream as loop0 fillers
        # everything else streams into pair0's PE gaps, one group per kt,
        # ordered so each V tile and K^T slice lands just before use
        f0 = [v_filler(2), v_filler(3), k2, v_filler(4), v_filler(5), k3,
              v_filler(6), v_filler(7), v_filler(8), v_filler(9),
              v_filler(10), v_filler(11), v_filler(12), v_filler(13),
              v_filler(14), v_filler(15)]
        attention_pair(0, fillers=f0, pace=1,
                       slow_fillers=[q2, q3] + proj_fillers(1))
        # wo load rides gpsimd's async SWDGE path after the startup-critical
        # window; needed only by pair-3-gated finals ~200us later
        nc.gpsimd.dma_start(out=wo_sb[:],
                            in_=wo.rearrange("(a p) n -> p a n", p=128))
        attention_pair(1, fillers=proj_fillers(2))
        attention_pair(2, fillers=proj_fillers(3))
        fin = [(lambda q: (lambda: emit_final([q])))(q) for q in range(8)]
        attention_pair(3, qc1_fillers=fin)
        emit_final(range(8, KT))

        if dbg:
            nc.sync.dma_start(out=d_qt[:], in_=qt_sb[0][:])
            nc.sync.dma_start(out=d_kt[:], in_=kt_sb[0][:])
            nc.sync.dma_start(out=d_v[:],
                              in_=v_sb[0][:].rearrange("p a b -> p (a b)"))
            nc.sync.dma_start(out=d_ot[:], in_=ot_sb[0][:])

    nc.compile()
    return nc


def get_program():
    if "nc" not in _CACHE:
        _CACHE["nc"] = _build_program()
    return _CACHE["nc"]


def _swap_c3(wo_slice):
    """Swap Wo's row blocks for head-pair chunk c=3: on the device, pair 3
    writes its SECOND head to ot rows 0:64 (direct, no dup on the critical
    tail) and its first head to rows 64:128."""
    w = np.array(wo_slice)
    w[3 * 128:3 * 128 + 64], w[3 * 128 + 64:4 * 128] = (
        w[3 * 128 + 64:4 * 128].copy(), w[3 * 128:3 * 128 + 64].copy())
    return w


def make_in_maps(inputs):
    dt = _np_mm_dtype()
    q = np.asarray(inputs["query"], np.float32)
    k = np.asarray(inputs["key"], np.float32)
    v = np.asarray(inputs["value"], np.float32)
    Wq = np.asarray(inputs["Wq"], np.float32)
    Wk = np.asarray(inputs["Wk"], np.float32)
    Wv = np.asarray(inputs["Wv"], np.float32)
    Wo = np.asarray(inputs["Wo"], np.float32)
    bq = np.asarray(inputs["bq"], np.float32)
    bk = np.asarray(inputs["bk"], np.float32)
    in_maps = []
    for core in range(NCORES):
        b, g = core // 2, core % 2
        sl = slice(g * GD, (g + 1) * GD)
        in_maps.append({
            "xqT": np.ascontiguousarray(q[b].T).astype(dt),
            "xkT": np.ascontiguousarray(k[b].T).astype(dt),
            "xvT": np.ascontiguousarray(v[b].T).astype(dt),
            "wq": np.ascontiguousarray(Wq[:, sl]).astype(dt),
            "wk": np.ascontiguousarray(Wk[:, sl]).astype(dt),
            "wv": np.ascontiguousarray(Wv[:, sl]).astype(dt),
            "wo": np.ascontiguousarray(_swap_c3(Wo[sl, :])).astype(dt),
            "bq": np.ascontiguousarray(bq[sl]),
            "bk": np.ascontiguousarray(bk[sl]),
        })
    return in_maps


def combine_outputs(results, inputs):
    Wo = np.asarray(inputs["Wo"], np.float32)
    bv = np.asarray(inputs["bv"], np.float32)
    bo = np.asarray(inputs["bo"], np.float32)
    out = np.empty((B, S, D), np.float32)
    for b in range(B):
        out[b] = (results[2 * b]["out"].astype(np.float32)
                  + results[2 * b + 1]["out"].astype(np.float32))
    out += bv @ Wo + bo
    return out


def kernel(**inputs):
    from concourse.bass_utils import run_bass_kernel_spmd
    nc = get_program()
    in_maps = make_in_maps(inputs)
    res = run_bass_kernel_spmd(nc, in_maps, list(range(NCORES)))
    return combine_outputs(res.results, inputs)



# revision 59
# speedup vs baseline: 1.0047x; 1.0047x over previous
"""Multi-head attention (B=4, S=2048, D=1024, H=16) on 8 trn2 NeuronCores.

Sharding: core = (batch b, head-group g) with b = core//2, g = core%2.
Each core handles one batch and 8 heads (512 of the 1024 d_model dims):
  - host pre-transposes query/key/value[b] -> [1024, 2048] so the device
    never transposes activations (and pre-casts to the matmul dtype)
  - device computes Q^T, K^T (head dims on partitions) and V (natural),
    attention with *transposed* scores S^T = K_h @ Q_h^T so softmax's
    denominator comes out of the PV matmul via a ones-column appended to V
  - output projection vs Wo[g*512:(g+1)*512, :] gives a partial [2048,1024]
  - host sums the two group partials per batch and adds bv@Wo + bo
Matmul operand dtype is MM_DT (bf16 default: full-rate PE streaming + FWL;
f32r fallback: fp22 multiplies at half stream rate). PSUM accumulation and
the softmax normalization chain stay fp32.
"""

import os
import numpy as np
from contextlib import ExitStack

B = 4
S = 2048
D = 1024
H = 16
DK = 64
NCORES = 8
GH = 8          # heads per core (group)
GD = GH * DK    # 512 head dims per core
NCH = GD // 128  # 4 chunks of 128 output dims
KT = S // 128    # 16 key tiles
QC = 1024        # q chunk width for attention
NQC = S // QC    # 2
SC = 512         # s chunk width for projections
NSC = S // SC    # 8
DMT = D // 128   # 8 d_model tiles

MM_DT = os.environ.get("MM_DT", "bf16")  # "bf16" | "f32r"

_CACHE = {}


def _np_mm_dtype():
    if MM_DT == "bf16":
        import ml_dtypes
        return ml_dtypes.bfloat16
    return np.float32


def _build_program():
    import concourse.mybir as mybir
    import concourse.tile as tile
    from concourse import bacc

    f32 = mybir.dt.float32
    f32r = mybir.dt.float32r
    dmm = mybir.dt.bfloat16 if MM_DT == "bf16" else mybir.dt.float32r

    nc = bacc.Bacc("TRN2", target_bir_lowering=False, debug=False,
                   num_devices=NCORES)

    xqT = nc.dram_tensor("xqT", [D, S], dmm, kind="ExternalInput").ap()
    xkT = nc.dram_tensor("xkT", [D, S], dmm, kind="ExternalInput").ap()
    xvT = nc.dram_tensor("xvT", [D, S], dmm, kind="ExternalInput").ap()
    wq = nc.dram_tensor("wq", [D, GD], dmm, kind="ExternalInput").ap()
    wk = nc.dram_tensor("wk", [D, GD], dmm, kind="ExternalInput").ap()
    wv = nc.dram_tensor("wv", [D, GD], dmm, kind="ExternalInput").ap()
    wo = nc.dram_tensor("wo", [GD, D], dmm, kind="ExternalInput").ap()
    bq = nc.dram_tensor("bq", [GD], f32, kind="ExternalInput").ap()
    bk = nc.dram_tensor("bk", [GD], f32, kind="ExternalInput").ap()
    out = nc.dram_tensor("out", [S, D], dmm, kind="ExternalOutput").ap()

    dbg = os.environ.get("DEBUG_DUMPS", "0") == "1"
    if dbg:
        d_qt = nc.dram_tensor("d_qt", [128, S], dmm, kind="ExternalOutput").ap()
        d_kt = nc.dram_tensor("d_kt", [128, S], dmm, kind="ExternalOutput").ap()
        d_v = nc.dram_tensor("d_v", [128, GH * 66], dmm,
                             kind="ExternalOutput").ap()
        d_pt = nc.dram_tensor("d_pt", [128, QC], dmm,
                              kind="ExternalOutput").ap()
        d_pv = nc.dram_tensor("d_pv", [65, QC], f32, kind="ExternalOutput").ap()
        d_zr = nc.dram_tensor("d_zr", [1, QC], f32, kind="ExternalOutput").ap()
        d_rb = nc.dram_tensor("d_rb", [DK, QC], f32, kind="ExternalOutput").ap()
        d_ot = nc.dram_tensor("d_ot", [128, S], dmm, kind="ExternalOutput").ap()

    Exp = mybir.ActivationFunctionType.Exp

    with tile.TileContext(nc) as tc, ExitStack() as ctx:
        # ---- pools (slots are statically reserved per tag) ----
        p_qt = ctx.enter_context(tc.tile_pool(name="qt", bufs=GH))
        p_kt = ctx.enter_context(tc.tile_pool(name="kt", bufs=GH))
        p_v = ctx.enter_context(tc.tile_pool(name="v", bufs=KT))
        p_ot = ctx.enter_context(tc.tile_pool(name="ot", bufs=NCH))
        p_wvo = ctx.enter_context(tc.tile_pool(name="wvo", bufs=1))
        p_wc = ctx.enter_context(tc.tile_pool(name="wc", bufs=1))
        p_bias = ctx.enter_context(tc.tile_pool(name="bias", bufs=1))
        p_xs = ctx.enter_context(tc.tile_pool(name="xs", bufs=3))
        p_pt = ctx.enter_context(tc.tile_pool(name="pt", bufs=10))
        p_zr = ctx.enter_context(tc.tile_pool(name="zr", bufs=2))
        p_rb = ctx.enter_context(tc.tile_pool(name="rb", bufs=2))
        p_ov = ctx.enter_context(tc.tile_pool(name="ov", bufs=2))
        p_st = ctx.enter_context(tc.tile_pool(name="st", bufs=3))
        # PSUM (8 banks total): scores 2x[128,1024]f32 (4) + proj
        # 2x[128,512]f32 (2) + PV accum 1x[65,1024]f32 (2). A separate proj
        # pool keeps scores-psum recycling off the DVE bias-add chain.
        p_ps = ctx.enter_context(tc.tile_pool(name="ps", bufs=2, space="PSUM"))
        p_pj = ctx.enter_context(tc.tile_pool(name="pj", bufs=2, space="PSUM"))
        p_pv = ctx.enter_context(tc.tile_pool(name="pv", bufs=1, space="PSUM"))

        # ---- biases + ones ----
        bq_sb = p_bias.tile([128, NCH], f32, tag="bq")
        nc.sync.dma_start(out=bq_sb[:], in_=bq.rearrange("(a p) -> p a", p=128))
        bk_sb = p_bias.tile([128, NCH], f32, tag="bk")
        nc.sync.dma_start(out=bk_sb[:], in_=bk.rearrange("(a p) -> p a", p=128))
        ones_sb = p_bias.tile([128, 1], f32, tag="ones")
        nc.vector.memset(ones_sb[:], 1.0)


        # ---- V projection: V_sb[st] = [128 s, GH, 65] (col 64 = ones) ----
        v_sb = []

        vstate = {}

        def v_filler(st):
            def emit():
                emit_v_st(st)
            return emit

        def emit_v_proj(first=0):
          wv_sb = p_wvo.tile([128, DMT, GD], dmm, tag="wvo", name="wv_sb")
          nc.scalar.dma_start(out=wv_sb[:],
                              in_=wv.rearrange("(a p) d -> p a d", p=128))
          vstate["wv"] = wv_sb
          for st in range(first):
              emit_v_st(st)

        def emit_v_st(st):
          wv_sb = vstate["wv"]
          if True:
              if st % 4 == 0:
                  xv_t = p_xs.tile([128, DMT, 512], dmm, tag="xs",
                                   name=f"xv{st}")
                  nc.sync.dma_start(
                      out=xv_t[:],
                      in_=xvT[:, st * 128:st * 128 + 512].rearrange(
                          "(a p) s -> p a s", p=128),
                  )
                  vstate["xv"] = xv_t
              xv_t = vstate["xv"]
              sub = (st % 4) * 128
              ps = p_pj.tile([128, GD], f32, tag="pj", name=f"psv{st}")
              for a in range(DMT):
                  nc.tensor.matmul(
                      out=ps[:],
                      lhsT=xv_t[:, a, sub:sub + 128],
                      rhs=wv_sb[:, a, :],
                      start=(a == 0), stop=(a == DMT - 1),
                  )
              # layout [ones | V(64) | ones]: even heads take cols 1:66
              # (Z lands at psum row 64), odd heads cols 0:65 written at
              # psum rows 63:128 (Z at row 63) -> both land row-aligned
              # with their ot half, so normalize needs no partition shift
              vt = p_v.tile([128, GH, 66], dmm, tag="v", name=f"v{st}")
              nc.vector.tensor_copy(
                  out=vt[:, :, 1:1 + DK],
                  in_=ps[:].rearrange("p (h d) -> p h d", h=GH),
              )
              nc.vector.tensor_copy(
                  out=vt[:, :, 0:1],
                  in_=ones_sb.unsqueeze(1).broadcast_to([128, GH, 1]))
              nc.vector.tensor_copy(
                  out=vt[:, :, 65:66],
                  in_=ones_sb.unsqueeze(1).broadcast_to([128, GH, 1]))
              v_sb.append(vt)

        qt_sb = [None] * GH
        kt_sb = [None] * GH
        ot_sb = [None] * NCH
        # deferred trailing PVs + normalize of the previous (qc,hh) loop;
        # they re-emit after the NEXT loop's first scores (cross-pair too)
        pend = {"d": []}

        wq_sb = p_wc.tile([128, DMT, GD], dmm, tag="wqc", name="wq_sb")
        nc.scalar.dma_start(out=wq_sb[:],
                          in_=wq.rearrange("(a p) d -> p a d", p=128))
        wk_sb = p_wc.tile([128, DMT, GD], dmm, tag="wkc", name="wk_sb")
        nc.scalar.dma_start(out=wk_sb[:],
                          in_=wk.rearrange("(a p) d -> p a d", p=128))

        def proj_fillers(c):
            """Emission groups computing per-head Q^T/K^T for heads 2c,2c+1.
            Each head tile [128, S] holds its 64 dims twice (rows 0-63 and
            64-127) so consecutive kt score matmuls alternate PE row groups
            and run concurrently."""
            for hh in range(2):
                hg = 2 * c + hh
                qt_sb[hg] = p_qt.tile([128, S], dmm, tag="qt", name=f"qt{hg}")
                kt_sb[hg] = p_kt.tile([128, S], dmm, tag="kt", name=f"kt{hg}")

            def group(src, wsb, bsb, dsts, nm, sc):
                def emit():
                    xs = p_xs.tile([128, DMT, SC], dmm, tag="xs",
                                   name=f"xs{nm}{c}_{sc}")
                    # k-chunks on the sync queue, q-chunks on gpsimd: the
                    # startup-critical loads stream on separate queues
                    dq = nc.sync if nm == "k" else nc.gpsimd
                    dq.dma_start(
                        out=xs[:],
                        in_=src[:, sc * SC:(sc + 1) * SC].rearrange(
                            "(a p) s -> p a s", p=128),
                    )
                    ps = p_pj.tile([128, SC], f32, tag="pj",
                                   name=f"psp{nm}{c}_{sc}")
                    for a in range(DMT):
                        nc.tensor.matmul(
                            out=ps[:],
                            lhsT=wsb[:, a, c * 128:(c + 1) * 128],
                            rhs=xs[:, a, :],
                            start=(a == 0), stop=(a == DMT - 1),
                        )
                    s0, s1 = sc * SC, (sc + 1) * SC
                    # head 2c native rows 0-63; head 2c+1 native rows 64-127
                    nc.vector.tensor_scalar_add(
                        out=dsts[0][0:DK, s0:s1], in0=ps[0:DK, :],
                        scalar1=bsb[0:DK, c:c + 1])
                    nc.vector.tensor_scalar_add(
                        out=dsts[1][DK:128, s0:s1], in0=ps[DK:128, :],
                        scalar1=bsb[DK:128, c:c + 1])
                    # duplicate this slice into the other half right away
                    # (SBUF->SBUF DMA) so scores kt for this s-range unblock
                    nc.sync.dma_start(out=dsts[0][DK:128, s0:s1],
                                      in_=dsts[0][0:DK, s0:s1])
                    nc.sync.dma_start(out=dsts[1][0:DK, s0:s1],
                                      in_=dsts[1][DK:128, s0:s1])
                return emit

            qd = [qt_sb[2 * c], qt_sb[2 * c + 1]]
            kd = [kt_sb[2 * c], kt_sb[2 * c + 1]]
            q = lambda sc: group(xqT, wq_sb, bq_sb, qd, "q", sc)
            k = lambda sc: group(xkT, wk_sb, bk_sb, kd, "k", sc)
            # K sc0 + Q sc0/1 first: pair c's scores kt=0 needs them
            return [k(0), q(0), q(1), k(1), k(2), q(2), k(3), q(3)]

        def attention_pair(c, fillers=(), pace=4, slow_fillers=(),
                           qc1_fillers=(), fin_cb=None):
            """Heads 2c, 2c+1 -> normalized O^T chunk c [128 dout, S].
            fillers: emission callbacks interleaved into the kt loop so
            next-chunk projections share PE/PSUM without starving ACT."""
            fillers = list(fillers)
            slow_fillers = list(slow_fillers)
            qc1_fillers = list(qc1_fillers)

            def normalize_half(qc, hh, pv_ps, half, direct):
                """per-512-half normalize chain (for the last loop, so
                dependent final projections can start after each half)"""
                sl = slice(half * 512, (half + 1) * 512)
                osl = slice(qc * QC + half * 512, qc * QC + (half + 1) * 512)
                ov = p_ov.tile([65, QC], f32, tag="ov",
                               name=f"ovh{c}_{qc}_{hh}_{half}")
                nc.vector.tensor_copy(out=ov[:, 0:512], in_=pv_ps[0:65, sl])
                zs = p_zr.tile([DK, QC // DK], f32, tag="zs",
                               name=f"zsh{c}_{qc}_{hh}_{half}")
                nc.sync.dma_start(out=zs[:, 0:8], in_=ov[DK:DK + 1, 0:512])
                nc.vector.reciprocal(out=zs[:, 0:8], in_=zs[:, 0:8])
                zr = p_zr.tile([1, QC], f32, tag="zr",
                               name=f"zrh{c}_{qc}_{hh}_{half}")
                nc.sync.dma_start(out=zr[:, 0:512], in_=zs[:, 0:8])
                rb = p_rb.tile([DK, QC], f32, tag="rb",
                               name=f"rbh{c}_{qc}_{hh}_{half}")
                nc.gpsimd.partition_broadcast(rb[:, 0:512], zr[:, 0:512],
                                              channels=DK)
                if direct:
                    nc.vector.tensor_mul(out=ot_sb[c][0:DK, osl],
                                         in0=ov[0:DK, 0:512],
                                         in1=rb[:, 0:512])
                else:
                    tmp = p_rb.tile([DK, QC], dmm, tag="tmp",
                                    name=f"tmph{c}_{qc}_{hh}_{half}")
                    nc.vector.tensor_mul(out=tmp[:, 0:512],
                                         in0=ov[0:DK, 0:512],
                                         in1=rb[:, 0:512])
                    nc.sync.dma_start(out=ot_sb[c][DK:128, osl],
                                      in_=tmp[:, 0:512])
            def normalize_full(qc, hh, pv_ps, direct):
                # normalize: DVE evicts PV psum (frees the bank pair
                # fast), takes 1/Z via scatter (64-wide; single-lane DVE
                # recip is ~6x slower), gpsimd broadcasts, DVE multiplies
                ov = p_ov.tile([65, QC], f32, tag="ov",
                               name=f"ov{c}_{qc}_{hh}")
                nc.vector.tensor_copy(out=ov[:], in_=pv_ps[0:65, :])
                zs = p_zr.tile([DK, QC // DK], f32, tag="zs",
                               name=f"zs{c}_{qc}_{hh}")
                nc.sync.dma_start(out=zs[:], in_=ov[DK:DK + 1, :])
                nc.vector.reciprocal(out=zs[:], in_=zs[:])
                zr = p_zr.tile([1, QC], f32, tag="zr",
                               name=f"zr{c}_{qc}_{hh}")
                nc.sync.dma_start(out=zr[:], in_=zs[:])
                rb = p_rb.tile([DK, QC], f32, tag="rb",
                               name=f"rb{c}_{qc}_{hh}")
                nc.gpsimd.partition_broadcast(rb[:], zr[:], channels=DK)
                if direct:
                    nc.vector.tensor_mul(
                        out=ot_sb[c][0:DK, qc * QC:(qc + 1) * QC],
                        in0=ov[0:DK, :], in1=rb[:])
                else:
                    tmp = p_rb.tile([DK, QC], dmm, tag="tmp",
                                    name=f"tmp{c}_{qc}")
                    nc.vector.tensor_mul(out=tmp[:], in0=ov[0:DK, :],
                                         in1=rb[:])
                    nc.sync.dma_start(
                        out=ot_sb[c][DK:128, qc * QC:(qc + 1) * QC],
                        in_=tmp[:])

            ot_sb[c] = p_ot.tile([128, S], dmm, tag="ot", name=f"ot{c}")
            for qc in range(NQC):
                for hh in range(2):
                    hg = 2 * c + hh
                    # PV -> psum rows 0:65 (num 0:64, Z at row 64). One head
                    # of the pair writes ot rows 0:64 directly; the other is
                    # shifted to rows 64:128 by an SBUF DMA. For pair 3 the
                    # hh processed LAST gets the direct write (no dup on the
                    # kernel's critical tail) -- the host swaps Wo's c=3 row
                    # blocks to match.
                    direct = (hh == 1) if c == 3 else (hh == 0)
                    last = fin_cb is not None and hh == 1 and qc == NQC - 1
                    pv_ps = p_pv.tile([128, QC], f32, tag="pv",
                                      name=f"pv{c}_{qc}_{hh}")

                    def emit_se(kt_i, qc=qc, hh=hh, hg=hg):
                        """scores + exp for kt_i -> P^T tile."""
                        rg = DK * (kt_i % 2)
                        ps = p_ps.tile([128, QC], f32, tag="ps",
                                       name=f"pss{c}_{qc}_{kt_i}_{hh}")
                        for half in range(QC // 512):
                            q0 = qc * QC + half * 512
                            nc.tensor.matmul(
                                out=ps[:, half * 512:(half + 1) * 512],
                                lhsT=kt_sb[hg][rg:rg + DK,
                                               kt_i * 128:(kt_i + 1) * 128],
                                rhs=qt_sb[hg][rg:rg + DK, q0:q0 + 512],
                                start=True, stop=True,
                            )
                        pt = p_pt.tile([128, QC], dmm, tag="pt",
                                       name=f"pt{c}_{qc}_{kt_i}_{hh}")
                        nc.scalar.activation(pt[:], ps[:], Exp,
                                             bias=0.0, scale=0.125)
                        return pt

                    # software pipeline: scores/exp run PIPE kts ahead of
                    # PV; the previous loop's trailing PVs + normalize are
                    # deferred until after this loop's first scores so the
                    # exp stream is seamless across loop boundaries
                    PIPE = 2
                    pts = [emit_se(i) for i in range(PIPE)]
                    for d_ in pend["d"]:
                        d_()
                    pend["d"] = []

                    def emit_pv(kt_i, pv_ps=pv_ps, hg=hg, pts=pts):
                        pt_cur = pts[kt_i]
                        for half in range(QC // 512):
                            nc.tensor.matmul(
                                out=pv_ps[0:65,
                                          half * 512:(half + 1) * 512],
                                lhsT=v_sb[kt_i][:, hg, 1:66],
                                rhs=pt_cur[:, half * 512:(half + 1) * 512],
                                start=(kt_i == 0), stop=(kt_i == KT - 1),
                            )

                    for kt_i in range(KT):
                        if kt_i + PIPE < KT:
                            pts.append(emit_se(kt_i + PIPE))
                        # pops at kt 1,5,9,12 -- never in the last 3 kts,
                        # where a filler would sit between this loop's tail
                        # PVs and the next loop's scores and drain the exp
                        # pipeline at every boundary
                        if fillers and (pace == 1 or kt_i in (1, 5, 9, 12)):
                            fillers.pop(0)()
                        elif slow_fillers and kt_i in (1, 5, 9, 12):
                            slow_fillers.pop(0)()
                        elif qc == 1 and qc1_fillers and kt_i in (1, 5, 9, 12):
                            qc1_fillers.pop(0)()
                        emit_pv(kt_i)
                    if last:
                        # last loop of the whole kernel: normalize per half
                        # so the trailing final projections start early
                        for half in range(QC // 512):
                            normalize_half(qc, hh, pv_ps, half, direct)
                            fin_cb(half)
                        continue
                    normalize_full(qc, hh, pv_ps, direct)


        # ---- output projection ----
        def emit_final(qts, use_act=False):
          Copy = mybir.ActivationFunctionType.Copy
          for qt_i in qts:
              st = p_st.tile([128, D], dmm, tag="st", name=f"st{qt_i}")
              if use_act:
                  # exp-free tail: borrow the idle scores pool (2-bank
                  # tiles) so 4 psum slots rotate, and let ACT drain them
                  ps = p_ps.tile([128, 1024], f32, tag="ps",
                                 name=f"pso{qt_i}")
                  for half in range(2):
                      sl = slice(half * 512, (half + 1) * 512)
                      for c in range(NCH):
                          nc.tensor.matmul(
                              out=ps[:, sl],
                              lhsT=ot_sb[c][:, qt_i * 128:(qt_i + 1) * 128],
                              rhs=wo_sb[:, c, sl],
                              start=(c == 0), stop=(c == NCH - 1),
                          )
                      nc.scalar.activation(st[:, sl], ps[:, sl], Copy)
              else:
                  for half in range(2):
                      sl = slice(half * 512, (half + 1) * 512)
                      ps = p_pj.tile([128, 512], f32, tag="pj",
                                     name=f"pso{qt_i}_{half}")
                      for c in range(NCH):
                          nc.tensor.matmul(
                              out=ps[:],
                              lhsT=ot_sb[c][:, qt_i * 128:(qt_i + 1) * 128],
                              rhs=wo_sb[:, c, sl],
                              start=(c == 0), stop=(c == NCH - 1),
                          )
                      nc.vector.tensor_copy(out=st[:, sl], in_=ps[:])
              nc.sync.dma_start(out=out[qt_i * 128:(qt_i + 1) * 128, :],
                                in_=st[:])

        # ---- emit: QK chunk 0 first so attention starts ASAP; V proj
        # streams in behind it; later chunk projections fill PE gaps ----
        wo_sb = p_wvo.tile([128, NCH, D], dmm, tag="wo", name="wo_sb")
        g0 = proj_fillers(0)
        for g in g0[:3]:        # k0, q0, q1 -> first scores ready ASAP
            g()
        k1, k2, q2, k3, q3 = g0[3], g0[4], g0[5], g0[6], g0[7]
        k1()                    # loop0 scores reach kt4 ~4 exps in
        emit_v_proj(first=4)    # wv + v0-v3 fill the DMA-bound startup;
                                # v4+ stream as loop0 fillers, draining by
                                # kt13 so the loop0 boundary stays clean
        # everything else streams into pair0's PE gaps, one group per kt,
        # ordered so each V tile and K^T slice lands just before use
        f0 = [k2, v_filler(4), v_filler(5), k3, v_filler(6), v_filler(7),
              v_filler(8), v_filler(9), v_filler(10), v_filler(11),
              v_filler(12), v_filler(13), v_filler(14), v_filler(15)]
        attention_pair(0, fillers=f0, pace=1,
                       slow_fillers=[q2, q3] + proj_fillers(1))
        # wo load rides gpsimd's async SWDGE path after the startup-critical
        # window; needed only by pair-3-gated finals ~200us later
        nc.gpsimd.dma_start(out=wo_sb[:],
                            in_=wo.rearrange("(a p) n -> p a n", p=128))
        attention_pair(1, fillers=proj_fillers(2))
        attention_pair(2, fillers=proj_fillers(3))
        fin = [(lambda q: (lambda: emit_final([q])))(q) for q in range(8)]
        attention_pair(3, qc1_fillers=fin)
        emit_final(range(8, KT))

        if dbg:
            nc.sync.dma_start(out=d_qt[:], in_=qt_sb[0][:])
            nc.sync.dma_start(out=d_kt[:], in_=kt_sb[0][:])
            nc.sync.dma_start(out=d_v[:],
                              in_=v_sb[0][:].rearrange("p a b -> p (a b)"))
            nc.sync.dma_start(out=d_ot[:], in_=ot_sb[0][:])

    nc.compile()
    return nc


def get_program():
    if "nc" not in _CACHE:
        _CACHE["nc"] = _build_program()
    return _CACHE["nc"]


def _swap_c3(wo_slice):
    """Swap Wo's row blocks for head-pair chunk c=3: on the device, pair 3
    writes its SECOND head to ot rows 0:64 (direct, no dup on the critical
    tail) and its first head to rows 64:128."""
    w = np.array(wo_slice)
    w[3 * 128:3 * 128 + 64], w[3 * 128 + 64:4 * 128] = (
        w[3 * 128 + 64:4 * 128].copy(), w[3 * 128:3 * 128 + 64].copy())
    return w


def make_in_maps(inputs):
    dt = _np_mm_dtype()
    q = np.asarray(inputs["query"], np.float32)
    k = np.asarray(inputs["key"], np.float32)
    v = np.asarray(inputs["value"], np.float32)
    Wq = np.asarray(inputs["Wq"], np.float32)
    Wk = np.asarray(inputs["Wk"], np.float32)
    Wv = np.asarray(inputs["Wv"], np.float32)
    Wo = np.asarray(inputs["Wo"], np.float32)
    bq = np.asarray(inputs["bq"], np.float32)
    bk = np.asarray(inputs["bk"], np.float32)
    in_maps = []
    for core in range(NCORES):
        b, g = core // 2, core % 2
        sl = slice(g * GD, (g + 1) * GD)
        in_maps.append({
            "xqT": np.ascontiguousarray(q[b].T).astype(dt),
            "xkT": np.ascontiguousarray(k[b].T).astype(dt),
            "xvT": np.ascontiguousarray(v[b].T).astype(dt),
            "wq": np.ascontiguousarray(Wq[:, sl]).astype(dt),
            "wk": np.ascontiguousarray(Wk[:, sl]).astype(dt),
            "wv": np.ascontiguousarray(Wv[:, sl]).astype(dt),
            "wo": np.ascontiguousarray(_swap_c3(Wo[sl, :])).astype(dt),
            "bq": np.ascontiguousarray(bq[sl]),
            "bk": np.ascontiguousarray(bk[sl]),
        })
    return in_maps


def combine_outputs(results, inputs):
    Wo = np.asarray(inputs["Wo"], np.float32)
    bv = np.asarray(inputs["bv"], np.float32)
    bo = np.asarray(inputs["bo"], np.float32)
    out = np.empty((B, S, D), np.float32)
    for b in range(B):
        out[b] = (results[2 * b]["out"].astype(np.float32)
                  + results[2 * b + 1]["out"].astype(np.float32))
    out += bv @ Wo + bo
    return out


def kernel(**inputs):
    from concourse.bass_utils import run_bass_kernel_spmd
    nc = get_program()
    in_maps = make_in_maps(inputs)
    res = run_bass_kernel_spmd(nc, in_maps, list(range(NCORES)))
    return combine_outputs(res.results, inputs)



# revision 60
# speedup vs baseline: 1.0098x; 1.0051x over previous
"""Multi-head attention (B=4, S=2048, D=1024, H=16) on 8 trn2 NeuronCores.

Sharding: core = (batch b, head-group g) with b = core//2, g = core%2.
Each core handles one batch and 8 heads (512 of the 1024 d_model dims):
  - host pre-transposes query/key/value[b] -> [1024, 2048] so the device
    never transposes activations (and pre-casts to the matmul dtype)
  - device computes Q^T, K^T (head dims on partitions) and V (natural),
    attention with *transposed* scores S^T = K_h @ Q_h^T so softmax's
    denominator comes out of the PV matmul via a ones-column appended to V
  - output projection vs Wo[g*512:(g+1)*512, :] gives a partial [2048,1024]
  - host sums the two group partials per batch and adds bv@Wo + bo
Matmul operand dtype is MM_DT (bf16 default: full-rate PE streaming + FWL;
f32r fallback: fp22 multiplies at half stream rate). PSUM accumulation and
the softmax normalization chain stay fp32.
"""

import os
import numpy as np
from contextlib import ExitStack

B = 4
S = 2048
D = 1024
H = 16
DK = 64
NCORES = 8
GH = 8          # heads per core (group)
GD = GH * DK    # 512 head dims per core
NCH = GD // 128  # 4 chunks of 128 output dims
KT = S // 128    # 16 key tiles
QC = 1024        # q chunk width for attention
NQC = S // QC    # 2
SC = 512         # s chunk width for projections
NSC = S // SC    # 8
DMT = D // 128   # 8 d_model tiles

MM_DT = os.environ.get("MM_DT", "bf16")  # "bf16" | "f32r"

_CACHE = {}


def _np_mm_dtype():
    if MM_DT == "bf16":
        import ml_dtypes
        return ml_dtypes.bfloat16
    return np.float32


def _build_program():
    import concourse.mybir as mybir
    import concourse.tile as tile
    from concourse import bacc

    f32 = mybir.dt.float32
    f32r = mybir.dt.float32r
    dmm = mybir.dt.bfloat16 if MM_DT == "bf16" else mybir.dt.float32r

    nc = bacc.Bacc("TRN2", target_bir_lowering=False, debug=False,
                   num_devices=NCORES)

    xqT = nc.dram_tensor("xqT", [D, S], dmm, kind="ExternalInput").ap()
    xkT = nc.dram_tensor("xkT", [D, S], dmm, kind="ExternalInput").ap()
    xvT = nc.dram_tensor("xvT", [D, S], dmm, kind="ExternalInput").ap()
    wq = nc.dram_tensor("wq", [D, GD], dmm, kind="ExternalInput").ap()
    wk = nc.dram_tensor("wk", [D, GD], dmm, kind="ExternalInput").ap()
    wv = nc.dram_tensor("wv", [D, GD], dmm, kind="ExternalInput").ap()
    wo = nc.dram_tensor("wo", [GD, D], dmm, kind="ExternalInput").ap()
    bq = nc.dram_tensor("bq", [GD], f32, kind="ExternalInput").ap()
    bk = nc.dram_tensor("bk", [GD], f32, kind="ExternalInput").ap()
    out = nc.dram_tensor("out", [S, D], dmm, kind="ExternalOutput").ap()

    dbg = os.environ.get("DEBUG_DUMPS", "0") == "1"
    if dbg:
        d_qt = nc.dram_tensor("d_qt", [128, S], dmm, kind="ExternalOutput").ap()
        d_kt = nc.dram_tensor("d_kt", [128, S], dmm, kind="ExternalOutput").ap()
        d_v = nc.dram_tensor("d_v", [128, GH * 66], dmm,
                             kind="ExternalOutput").ap()
        d_pt = nc.dram_tensor("d_pt", [128, QC], dmm,
                              kind="ExternalOutput").ap()
        d_pv = nc.dram_tensor("d_pv", [65, QC], f32, kind="ExternalOutput").ap()
        d_zr = nc.dram_tensor("d_zr", [1, QC], f32, kind="ExternalOutput").ap()
        d_rb = nc.dram_tensor("d_rb", [DK, QC], f32, kind="ExternalOutput").ap()
        d_ot = nc.dram_tensor("d_ot", [128, S], dmm, kind="ExternalOutput").ap()

    Exp = mybir.ActivationFunctionType.Exp

    with tile.TileContext(nc) as tc, ExitStack() as ctx:
        # ---- pools (slots are statically reserved per tag) ----
        p_qt = ctx.enter_context(tc.tile_pool(name="qt", bufs=GH))
        p_kt = ctx.enter_context(tc.tile_pool(name="kt", bufs=GH))
        p_v = ctx.enter_context(tc.tile_pool(name="v", bufs=KT))
        p_ot = ctx.enter_context(tc.tile_pool(name="ot", bufs=NCH))
        p_wvo = ctx.enter_context(tc.tile_pool(name="wvo", bufs=1))
        p_wc = ctx.enter_context(tc.tile_pool(name="wc", bufs=1))
        p_bias = ctx.enter_context(tc.tile_pool(name="bias", bufs=1))
        p_xs = ctx.enter_context(tc.tile_pool(name="xs", bufs=3))
        p_pt = ctx.enter_context(tc.tile_pool(name="pt", bufs=10))
        p_zr = ctx.enter_context(tc.tile_pool(name="zr", bufs=2))
        p_rb = ctx.enter_context(tc.tile_pool(name="rb", bufs=2))
        p_ov = ctx.enter_context(tc.tile_pool(name="ov", bufs=2))
        p_st = ctx.enter_context(tc.tile_pool(name="st", bufs=3))
        # PSUM (8 banks total): scores 2x[128,1024]f32 (4) + proj
        # 2x[128,512]f32 (2) + PV accum 1x[65,1024]f32 (2). A separate proj
        # pool keeps scores-psum recycling off the DVE bias-add chain.
        p_ps = ctx.enter_context(tc.tile_pool(name="ps", bufs=2, space="PSUM"))
        p_pj = ctx.enter_context(tc.tile_pool(name="pj", bufs=2, space="PSUM"))
        p_pv = ctx.enter_context(tc.tile_pool(name="pv", bufs=1, space="PSUM"))

        # ---- biases + ones ----
        bq_sb = p_bias.tile([128, NCH], f32, tag="bq")
        nc.sync.dma_start(out=bq_sb[:], in_=bq.rearrange("(a p) -> p a", p=128))
        bk_sb = p_bias.tile([128, NCH], f32, tag="bk")
        nc.sync.dma_start(out=bk_sb[:], in_=bk.rearrange("(a p) -> p a", p=128))
        ones_sb = p_bias.tile([128, 1], f32, tag="ones")
        nc.vector.memset(ones_sb[:], 1.0)


        # ---- V projection: V_sb[st] = [128 s, GH, 65] (col 64 = ones) ----
        v_sb = []

        vstate = {}

        def v_filler(st):
            def emit():
                emit_v_st(st)
            return emit

        def emit_v_proj(first=0):
          wv_sb = p_wvo.tile([128, DMT, GD], dmm, tag="wvo", name="wv_sb")
          nc.scalar.dma_start(out=wv_sb[:],
                              in_=wv.rearrange("(a p) d -> p a d", p=128))
          vstate["wv"] = wv_sb
          for st in range(first):
              emit_v_st(st)

        def emit_v_st(st):
          wv_sb = vstate["wv"]
          if True:
              if st % 4 == 0:
                  xv_t = p_xs.tile([128, DMT, 512], dmm, tag="xs",
                                   name=f"xv{st}")
                  # gpsimd, behind the q-chunks: keeps the sync queue free
                  # for the startup qt/kt duplication DMAs that gate the
                  # second scores tile
                  nc.gpsimd.dma_start(
                      out=xv_t[:],
                      in_=xvT[:, st * 128:st * 128 + 512].rearrange(
                          "(a p) s -> p a s", p=128),
                  )
                  vstate["xv"] = xv_t
              xv_t = vstate["xv"]
              sub = (st % 4) * 128
              ps = p_pj.tile([128, GD], f32, tag="pj", name=f"psv{st}")
              for a in range(DMT):
                  nc.tensor.matmul(
                      out=ps[:],
                      lhsT=xv_t[:, a, sub:sub + 128],
                      rhs=wv_sb[:, a, :],
                      start=(a == 0), stop=(a == DMT - 1),
                  )
              # layout [ones | V(64) | ones]: even heads take cols 1:66
              # (Z lands at psum row 64), odd heads cols 0:65 written at
              # psum rows 63:128 (Z at row 63) -> both land row-aligned
              # with their ot half, so normalize needs no partition shift
              vt = p_v.tile([128, GH, 66], dmm, tag="v", name=f"v{st}")
              nc.vector.tensor_copy(
                  out=vt[:, :, 1:1 + DK],
                  in_=ps[:].rearrange("p (h d) -> p h d", h=GH),
              )
              nc.vector.tensor_copy(
                  out=vt[:, :, 0:1],
                  in_=ones_sb.unsqueeze(1).broadcast_to([128, GH, 1]))
              nc.vector.tensor_copy(
                  out=vt[:, :, 65:66],
                  in_=ones_sb.unsqueeze(1).broadcast_to([128, GH, 1]))
              v_sb.append(vt)

        qt_sb = [None] * GH
        kt_sb = [None] * GH
        ot_sb = [None] * NCH
        # deferred trailing PVs + normalize of the previous (qc,hh) loop;
        # they re-emit after the NEXT loop's first scores (cross-pair too)
        pend = {"d": []}

        wq_sb = p_wc.tile([128, DMT, GD], dmm, tag="wqc", name="wq_sb")
        nc.scalar.dma_start(out=wq_sb[:],
                          in_=wq.rearrange("(a p) d -> p a d", p=128))
        wk_sb = p_wc.tile([128, DMT, GD], dmm, tag="wkc", name="wk_sb")
        nc.scalar.dma_start(out=wk_sb[:],
                          in_=wk.rearrange("(a p) d -> p a d", p=128))

        def proj_fillers(c):
            """Emission groups computing per-head Q^T/K^T for heads 2c,2c+1.
            Each head tile [128, S] holds its 64 dims twice (rows 0-63 and
            64-127) so consecutive kt score matmuls alternate PE row groups
            and run concurrently."""
            for hh in range(2):
                hg = 2 * c + hh
                qt_sb[hg] = p_qt.tile([128, S], dmm, tag="qt", name=f"qt{hg}")
                kt_sb[hg] = p_kt.tile([128, S], dmm, tag="kt", name=f"kt{hg}")

            def group(src, wsb, bsb, dsts, nm, sc):
                def emit():
                    xs = p_xs.tile([128, DMT, SC], dmm, tag="xs",
                                   name=f"xs{nm}{c}_{sc}")
                    # k-chunks on the sync queue, q-chunks on gpsimd: the
                    # startup-critical loads stream on separate queues
                    dq = nc.sync if nm == "k" else nc.gpsimd
                    dq.dma_start(
                        out=xs[:],
                        in_=src[:, sc * SC:(sc + 1) * SC].rearrange(
                            "(a p) s -> p a s", p=128),
                    )
                    ps = p_pj.tile([128, SC], f32, tag="pj",
                                   name=f"psp{nm}{c}_{sc}")
                    for a in range(DMT):
                        nc.tensor.matmul(
                            out=ps[:],
                            lhsT=wsb[:, a, c * 128:(c + 1) * 128],
                            rhs=xs[:, a, :],
                            start=(a == 0), stop=(a == DMT - 1),
                        )
                    s0, s1 = sc * SC, (sc + 1) * SC
                    # head 2c native rows 0-63; head 2c+1 native rows 64-127
                    nc.vector.tensor_scalar_add(
                        out=dsts[0][0:DK, s0:s1], in0=ps[0:DK, :],
                        scalar1=bsb[0:DK, c:c + 1])
                    nc.vector.tensor_scalar_add(
                        out=dsts[1][DK:128, s0:s1], in0=ps[DK:128, :],
                        scalar1=bsb[DK:128, c:c + 1])
                    # duplicate this slice into the other half right away
                    # (SBUF->SBUF DMA) so scores kt for this s-range unblock
                    nc.sync.dma_start(out=dsts[0][DK:128, s0:s1],
                                      in_=dsts[0][0:DK, s0:s1])
                    nc.sync.dma_start(out=dsts[1][0:DK, s0:s1],
                                      in_=dsts[1][DK:128, s0:s1])
                return emit

            qd = [qt_sb[2 * c], qt_sb[2 * c + 1]]
            kd = [kt_sb[2 * c], kt_sb[2 * c + 1]]
            q = lambda sc: group(xqT, wq_sb, bq_sb, qd, "q", sc)
            k = lambda sc: group(xkT, wk_sb, bk_sb, kd, "k", sc)
            # K sc0 + Q sc0/1 first: pair c's scores kt=0 needs them
            return [k(0), q(0), q(1), k(1), k(2), q(2), k(3), q(3)]

        def attention_pair(c, fillers=(), pace=4, slow_fillers=(),
                           qc1_fillers=(), fin_cb=None):
            """Heads 2c, 2c+1 -> normalized O^T chunk c [128 dout, S].
            fillers: emission callbacks interleaved into the kt loop so
            next-chunk projections share PE/PSUM without starving ACT."""
            fillers = list(fillers)
            slow_fillers = list(slow_fillers)
            qc1_fillers = list(qc1_fillers)

            def normalize_half(qc, hh, pv_ps, half, direct):
                """per-512-half normalize chain (for the last loop, so
                dependent final projections can start after each half)"""
                sl = slice(half * 512, (half + 1) * 512)
                osl = slice(qc * QC + half * 512, qc * QC + (half + 1) * 512)
                ov = p_ov.tile([65, QC], f32, tag="ov",
                               name=f"ovh{c}_{qc}_{hh}_{half}")
                nc.vector.tensor_copy(out=ov[:, 0:512], in_=pv_ps[0:65, sl])
                zs = p_zr.tile([DK, QC // DK], f32, tag="zs",
                               name=f"zsh{c}_{qc}_{hh}_{half}")
                nc.sync.dma_start(out=zs[:, 0:8], in_=ov[DK:DK + 1, 0:512])
                nc.vector.reciprocal(out=zs[:, 0:8], in_=zs[:, 0:8])
                zr = p_zr.tile([1, QC], f32, tag="zr",
                               name=f"zrh{c}_{qc}_{hh}_{half}")
                nc.sync.dma_start(out=zr[:, 0:512], in_=zs[:, 0:8])
                rb = p_rb.tile([DK, QC], f32, tag="rb",
                               name=f"rbh{c}_{qc}_{hh}_{half}")
                nc.gpsimd.partition_broadcast(rb[:, 0:512], zr[:, 0:512],
                                              channels=DK)
                if direct:
                    nc.vector.tensor_mul(out=ot_sb[c][0:DK, osl],
                                         in0=ov[0:DK, 0:512],
                                         in1=rb[:, 0:512])
                else:
                    tmp = p_rb.tile([DK, QC], dmm, tag="tmp",
                                    name=f"tmph{c}_{qc}_{hh}_{half}")
                    nc.vector.tensor_mul(out=tmp[:, 0:512],
                                         in0=ov[0:DK, 0:512],
                                         in1=rb[:, 0:512])
                    nc.sync.dma_start(out=ot_sb[c][DK:128, osl],
                                      in_=tmp[:, 0:512])
            def normalize_full(qc, hh, pv_ps, direct):
                # normalize: DVE evicts PV psum (frees the bank pair
                # fast), takes 1/Z via scatter (64-wide; single-lane DVE
                # recip is ~6x slower), gpsimd broadcasts, DVE multiplies
                ov = p_ov.tile([65, QC], f32, tag="ov",
                               name=f"ov{c}_{qc}_{hh}")
                nc.vector.tensor_copy(out=ov[:], in_=pv_ps[0:65, :])
                zs = p_zr.tile([DK, QC // DK], f32, tag="zs",
                               name=f"zs{c}_{qc}_{hh}")
                nc.sync.dma_start(out=zs[:], in_=ov[DK:DK + 1, :])
                nc.vector.reciprocal(out=zs[:], in_=zs[:])
                zr = p_zr.tile([1, QC], f32, tag="zr",
                               name=f"zr{c}_{qc}_{hh}")
                nc.sync.dma_start(out=zr[:], in_=zs[:])
                rb = p_rb.tile([DK, QC], f32, tag="rb",
                               name=f"rb{c}_{qc}_{hh}")
                nc.gpsimd.partition_broadcast(rb[:], zr[:], channels=DK)
                if direct:
                    nc.vector.tensor_mul(
                        out=ot_sb[c][0:DK, qc * QC:(qc + 1) * QC],
                        in0=ov[0:DK, :], in1=rb[:])
                else:
                    tmp = p_rb.tile([DK, QC], dmm, tag="tmp",
                                    name=f"tmp{c}_{qc}")
                    nc.vector.tensor_mul(out=tmp[:], in0=ov[0:DK, :],
                                         in1=rb[:])
                    nc.sync.dma_start(
                        out=ot_sb[c][DK:128, qc * QC:(qc + 1) * QC],
                        in_=tmp[:])

            ot_sb[c] = p_ot.tile([128, S], dmm, tag="ot", name=f"ot{c}")
            for qc in range(NQC):
                for hh in range(2):
                    hg = 2 * c + hh
                    # PV -> psum rows 0:65 (num 0:64, Z at row 64). One head
                    # of the pair writes ot rows 0:64 directly; the other is
                    # shifted to rows 64:128 by an SBUF DMA. For pair 3 the
                    # hh processed LAST gets the direct write (no dup on the
                    # kernel's critical tail) -- the host swaps Wo's c=3 row
                    # blocks to match.
                    direct = (hh == 1) if c == 3 else (hh == 0)
                    last = fin_cb is not None and hh == 1 and qc == NQC - 1
                    pv_ps = p_pv.tile([128, QC], f32, tag="pv",
                                      name=f"pv{c}_{qc}_{hh}")

                    def emit_se(kt_i, qc=qc, hh=hh, hg=hg):
                        """scores + exp for kt_i -> P^T tile."""
                        rg = DK * (kt_i % 2)
                        ps = p_ps.tile([128, QC], f32, tag="ps",
                                       name=f"pss{c}_{qc}_{kt_i}_{hh}")
                        for half in range(QC // 512):
                            q0 = qc * QC + half * 512
                            nc.tensor.matmul(
                                out=ps[:, half * 512:(half + 1) * 512],
                                lhsT=kt_sb[hg][rg:rg + DK,
                                               kt_i * 128:(kt_i + 1) * 128],
                                rhs=qt_sb[hg][rg:rg + DK, q0:q0 + 512],
                                start=True, stop=True,
                            )
                        pt = p_pt.tile([128, QC], dmm, tag="pt",
                                       name=f"pt{c}_{qc}_{kt_i}_{hh}")
                        nc.scalar.activation(pt[:], ps[:], Exp,
                                             bias=0.0, scale=0.125)
                        return pt

                    # software pipeline: scores/exp run PIPE kts ahead of
                    # PV; the previous loop's trailing PVs + normalize are
                    # deferred until after this loop's first scores so the
                    # exp stream is seamless across loop boundaries
                    PIPE = 2
                    pts = [emit_se(i) for i in range(PIPE)]
                    for d_ in pend["d"]:
                        d_()
                    pend["d"] = []

                    def emit_pv(kt_i, pv_ps=pv_ps, hg=hg, pts=pts):
                        pt_cur = pts[kt_i]
                        for half in range(QC // 512):
                            nc.tensor.matmul(
                                out=pv_ps[0:65,
                                          half * 512:(half + 1) * 512],
                                lhsT=v_sb[kt_i][:, hg, 1:66],
                                rhs=pt_cur[:, half * 512:(half + 1) * 512],
                                start=(kt_i == 0), stop=(kt_i == KT - 1),
                            )

                    for kt_i in range(KT):
                        if kt_i + PIPE < KT:
                            pts.append(emit_se(kt_i + PIPE))
                        # pops at kt 1,5,9,12 -- never in the last 3 kts,
                        # where a filler would sit between this loop's tail
                        # PVs and the next loop's scores and drain the exp
                        # pipeline at every boundary
                        if fillers and (pace == 1 or kt_i in (1, 5, 9, 12)):
                            fillers.pop(0)()
                        elif slow_fillers and kt_i in (1, 5, 9, 12):
                            slow_fillers.pop(0)()
                        elif qc == 1 and qc1_fillers and kt_i in (1, 5, 9, 12):
                            qc1_fillers.pop(0)()
                        emit_pv(kt_i)
                    if last:
                        # last loop of the whole kernel: normalize per half
                        # so the trailing final projections start early
                        for half in range(QC // 512):
                            normalize_half(qc, hh, pv_ps, half, direct)
                            fin_cb(half)
                        continue
                    normalize_full(qc, hh, pv_ps, direct)


        # ---- output projection ----
        def emit_final(qts, use_act=False):
          Copy = mybir.ActivationFunctionType.Copy
          for qt_i in qts:
              st = p_st.tile([128, D], dmm, tag="st", name=f"st{qt_i}")
              if use_act:
                  # exp-free tail: borrow the idle scores pool (2-bank
                  # tiles) so 4 psum slots rotate, and let ACT drain them
                  ps = p_ps.tile([128, 1024], f32, tag="ps",
                                 name=f"pso{qt_i}")
                  for half in range(2):
                      sl = slice(half * 512, (half + 1) * 512)
                      for c in range(NCH):
                          nc.tensor.matmul(
                              out=ps[:, sl],
                              lhsT=ot_sb[c][:, qt_i * 128:(qt_i + 1) * 128],
                              rhs=wo_sb[:, c, sl],
                              start=(c == 0), stop=(c == NCH - 1),
                          )
                      nc.scalar.activation(st[:, sl], ps[:, sl], Copy)
              else:
                  for half in range(2):
                      sl = slice(half * 512, (half + 1) * 512)
                      ps = p_pj.tile([128, 512], f32, tag="pj",
                                     name=f"pso{qt_i}_{half}")
                      for c in range(NCH):
                          nc.tensor.matmul(
                              out=ps[:],
                              lhsT=ot_sb[c][:, qt_i * 128:(qt_i + 1) * 128],
                              rhs=wo_sb[:, c, sl],
                              start=(c == 0), stop=(c == NCH - 1),
                          )
                      nc.vector.tensor_copy(out=st[:, sl], in_=ps[:])
              nc.sync.dma_start(out=out[qt_i * 128:(qt_i + 1) * 128, :],
                                in_=st[:])

        # ---- emit: QK chunk 0 first so attention starts ASAP; V proj
        # streams in behind it; later chunk projections fill PE gaps ----
        wo_sb = p_wvo.tile([128, NCH, D], dmm, tag="wo", name="wo_sb")
        g0 = proj_fillers(0)
        for g in g0[:3]:        # k0, q0, q1 -> first scores ready ASAP
            g()
        k1, k2, q2, k3, q3 = g0[3], g0[4], g0[5], g0[6], g0[7]
        k1()                    # loop0 scores reach kt4 ~4 exps in
        emit_v_proj(first=4)    # wv + v0-v3 fill the DMA-bound startup;
                                # v4+ stream as loop0 fillers, draining by
                                # kt13 so the loop0 boundary stays clean
        # everything else streams into pair0's PE gaps, one group per kt,
        # ordered so each V tile and K^T slice lands just before use
        f0 = [k2, v_filler(4), v_filler(5), k3, v_filler(6), v_filler(7),
              v_filler(8), v_filler(9), v_filler(10), v_filler(11),
              v_filler(12), v_filler(13), v_filler(14), v_filler(15)]
        attention_pair(0, fillers=f0, pace=1,
                       slow_fillers=[q2, q3] + proj_fillers(1))
        # wo load rides gpsimd's async SWDGE path after the startup-critical
        # window; needed only by pair-3-gated finals ~200us later
        nc.gpsimd.dma_start(out=wo_sb[:],
                            in_=wo.rearrange("(a p) n -> p a n", p=128))
        attention_pair(1, fillers=proj_fillers(2))
        attention_pair(2, fillers=proj_fillers(3))
        fin = [(lambda q: (lambda: emit_final([q])))(q) for q in range(8)]
        attention_pair(3, qc1_fillers=fin)
        emit_final(range(8, KT))

        if dbg:
            nc.sync.dma_start(out=d_qt[:], in_=qt_sb[0][:])
            nc.sync.dma_start(out=d_kt[:], in_=kt_sb[0][:])
            nc.sync.dma_start(out=d_v[:],
                              in_=v_sb[0][:].rearrange("p a b -> p (a b)"))
            nc.sync.dma_start(out=d_ot[:], in_=ot_sb[0][:])

    nc.compile()
    return nc


def get_program():
    if "nc" not in _CACHE:
        _CACHE["nc"] = _build_program()
    return _CACHE["nc"]


def _swap_c3(wo_slice):
    """Swap Wo's row blocks for head-pair chunk c=3: on the device, pair 3
    writes its SECOND head to ot rows 0:64 (direct, no dup on the critical
    tail) and its first head to rows 64:128."""
    w = np.array(wo_slice)
    w[3 * 128:3 * 128 + 64], w[3 * 128 + 64:4 * 128] = (
        w[3 * 128 + 64:4 * 128].copy(), w[3 * 128:3 * 128 + 64].copy())
    return w


def make_in_maps(inputs):
    dt = _np_mm_dtype()
    q = np.asarray(inputs["query"], np.float32)
    k = np.asarray(inputs["key"], np.float32)
    v = np.asarray(inputs["value"], np.float32)
    Wq = np.asarray(inputs["Wq"], np.float32)
    Wk = np.asarray(inputs["Wk"], np.float32)
    Wv = np.asarray(inputs["Wv"], np.float32)
    Wo = np.asarray(inputs["Wo"], np.float32)
    bq = np.asarray(inputs["bq"], np.float32)
    bk = np.asarray(inputs["bk"], np.float32)
    in_maps = []
    for core in range(NCORES):
        b, g = core // 2, core % 2
        sl = slice(g * GD, (g + 1) * GD)
        in_maps.append({
            "xqT": np.ascontiguousarray(q[b].T).astype(dt),
            "xkT": np.ascontiguousarray(k[b].T).astype(dt),
            "xvT": np.ascontiguousarray(v[b].T).astype(dt),
            "wq": np.ascontiguousarray(Wq[:, sl]).astype(dt),
            "wk": np.ascontiguousarray(Wk[:, sl]).astype(dt),
            "wv": np.ascontiguousarray(Wv[:, sl]).astype(dt),
            "wo": np.ascontiguousarray(_swap_c3(Wo[sl, :])).astype(dt),
            "bq": np.ascontiguousarray(bq[sl]),
            "bk": np.ascontiguousarray(bk[sl]),
        })
    return in_maps


def combine_outputs(results, inputs):
    Wo = np.asarray(inputs["Wo"], np.float32)
    bv = np.asarray(inputs["bv"], np.float32)
    bo = np.asarray(inputs["bo"], np.float32)
    out = np.empty((B, S, D), np.float32)
    for b in range(B):
        out[b] = (results[2 * b]["out"].astype(np.float32)
                  + results[2 * b + 1]["out"].astype(np.float32))
    out += bv @ Wo + bo
    return out


def kernel(**inputs):
    from concourse.bass_utils import run_bass_kernel_spmd
    nc = get_program()
    in_maps = make_in_maps(inputs)
    res = run_bass_kernel_spmd(nc, in_maps, list(range(NCORES)))
    return combine_outputs(res.results, inputs)

